# revision 5
# baseline (speedup 1.0000x reference)
"""Trainium2 Bass kernel for segment-mean embedding-bag + 3-layer MLP.

Problem (hardcoded, from spec):
  emb_table [100000, 64] f32, feature_indices [819200] int, batch_indices
  [819200] int (sorted), W0..W2 [64,64], b0..b2 [64].
  out[s] = relu-MLP( mean_{i: batch_indices[i]==s} emb_table[feature_indices[i]] )

Strategy (8 NeuronCores, data-parallel over batch segments):
  - Each core owns 2048 contiguous segments (16 chunks x 128 segments).
  - Host prep (sharding): for each chunk, build a compact per-chunk table
    (unique rows referenced by that chunk, plus a zeros row for padding)
    and an int16 index list ordered so that gather position j = k*128 + p
    holds occurrence k of segment p.  dma_gather then lands segment p's
    rows in SBUF partition p, occurrences along the free axis.
  - Device: one dma_gather per chunk (128*KMAX rows of 256B), pairwise
    fold on DVE for the segment sum, multiply by 1/count, then the MLP on
    the tensor engine in transposed form (W_l stationary), ReLU+bias on
    the scalar engine, transpose back, DMA out.
"""

import numpy as np

VOCAB = 100000
DIMS = 64
B = 16384
N_CORES = 8
SEG_TILE = 128  # segments per chunk

_NC_CACHE: dict[tuple, object] = {}


# ----------------------------------------------------------------------------
# Host-side sharding / index preparation (numpy only)
# ----------------------------------------------------------------------------

def _host_prep(emb_table, W0, b0, W1, b1, W2, b2, feature_indices, batch_indices):
    emb = np.ascontiguousarray(np.asarray(emb_table, dtype=np.float32))
    fidx = np.asarray(feature_indices).astype(np.int64, copy=False)
    bidx = np.asarray(batch_indices).astype(np.int64, copy=False)
    nnz = fidx.shape[0]

    counts = np.bincount(bidx, minlength=B).astype(np.int64)
    starts = np.zeros(B + 1, dtype=np.int64)
    np.cumsum(counts, out=starts[1:])
    kmax = max(int(counts.max()), 1)

    # slot[s, k] = feature id of segment s's k-th occurrence, or -1 if k >= count
    ar = np.arange(kmax, dtype=np.int64)
    pos = starts[:-1, None] + np.minimum(ar[None, :], np.maximum(counts[:, None] - 1, 0))
    np.clip(pos, 0, max(nnz - 1, 0), out=pos)
    valid = ar[None, :] < counts[:, None]
    slot = np.where(valid, fidx[pos], -1)  # [B, kmax]

    b_loc = B // N_CORES
    n_chunks = b_loc // SEG_TILE
    npc = SEG_TILE * kmax          # gather rows per chunk
    fpc = npc // 16                # idx free-dim per chunk (int16 wrap)

    per_chunk = []  # (core, chunk) -> (rows [u,64] f32, idx16 [128, kmax])
    r_max = 0
    for core in range(N_CORES):
        for c in range(n_chunks):
            s0 = core * b_loc + c * SEG_TILE
            sm = slot[s0:s0 + SEG_TILE]  # [128, kmax]
            u, inv = np.unique(sm, return_inverse=True)
            inv = inv.reshape(sm.shape).astype(np.int16)
            if u[0] == -1:
                rows = np.empty((len(u), DIMS), dtype=np.float32)
                rows[0] = 0.0
                rows[1:] = emb[u[1:]]
            else:
                rows = emb[u]
            r_max = max(r_max, len(u))
            per_chunk.append((rows, inv))

    # stable table row-count across runs of the same input scale
    r_chunk = -(-r_max // 512) * 512

    in_maps = []
    wmat = np.ascontiguousarray(
        np.stack([W0, W1, W2]).astype(np.float32))          # [3, 64, 64]
    bmat = np.ascontiguousarray(
        np.stack([b0, b1, b2], axis=1).astype(np.float32))  # [64, 3]
    with np.errstate(divide="ignore"):
        recip_all = np.where(counts > 0, 1.0 / counts, np.inf).astype(np.float32)

    for core in range(N_CORES):
        table = np.zeros((n_chunks, r_chunk, DIMS), dtype=np.float32)
        idxs = np.empty((128, n_chunks * fpc), dtype=np.int16)
        for c in range(n_chunks):
            rows, inv = per_chunk[core * n_chunks + c]
            table[c, : len(rows)] = rows
            # position j = k*128 + p  ->  idx16[p, k];  wrap by 16, replicate x8
            arr = inv.T.ravel()                       # [npc]
            wrapped = arr.reshape(-1, 16).T           # [16, fpc]
            idxs[:, c * fpc:(c + 1) * fpc] = np.tile(wrapped, (8, 1))
        recip = np.ascontiguousarray(
            recip_all[core * b_loc:(core + 1) * b_loc].reshape(n_chunks, SEG_TILE).T
        )  # [128, n_chunks]
        in_maps.append({
            "table": table,
            "idxs": idxs,
            "recip": recip,
            "wmat": wmat,
            "bmat": bmat,
        })

    meta = (kmax, r_chunk, n_chunks)
    return in_maps, meta


# ----------------------------------------------------------------------------
# Bass program
# ----------------------------------------------------------------------------

def _build_nc(meta):
    if meta in _NC_CACHE:
        return _NC_CACHE[meta]

    import concourse.bacc as bacc
    import concourse.tile as tile
    from concourse import mybir
    from concourse.masks import make_identity

    kmax, r_chunk, n_chunks = meta
    npc = SEG_TILE * kmax
    fpc = npc // 16
    f32 = mybir.dt.float32
    i16 = mybir.dt.int16

    nc = bacc.Bacc("TRN2", target_bir_lowering=False, debug=False,
                   enable_asserts=False, num_devices=N_CORES)

    table = nc.dram_tensor("table", [n_chunks, r_chunk, DIMS], f32, kind="ExternalInput")
    idxs = nc.dram_tensor("idxs", [128, n_chunks * fpc], i16, kind="ExternalInput")
    recip = nc.dram_tensor("recip", [128, n_chunks], f32, kind="ExternalInput")
    wmat = nc.dram_tensor("wmat", [3, DIMS, DIMS], f32, kind="ExternalInput")
    bmat = nc.dram_tensor("bmat", [DIMS, 3], f32, kind="ExternalInput")
    out = nc.dram_tensor("out", [n_chunks * SEG_TILE, DIMS], f32, kind="ExternalOutput")

    with tile.TileContext(nc) as tc:
        with tc.tile_pool(name="const", bufs=1) as constp, \
             tc.tile_pool(name="gat", bufs=4) as gatp, \
             tc.tile_pool(name="work", bufs=2) as workp, \
             tc.tile_pool(name="ps", bufs=2, space="PSUM") as psump:

            idx_sb = constp.tile([128, n_chunks * fpc], i16, tag="idx")
            nc.sync.dma_start(out=idx_sb[:], in_=idxs[:])
            recip_sb = constp.tile([128, n_chunks], f32, tag="recip")
            nc.sync.dma_start(out=recip_sb[:], in_=recip[:])
            w_sb = []
            for l in range(3):
                w = constp.tile([DIMS, DIMS], f32, tag=f"w{l}")
                nc.sync.dma_start(out=w[:], in_=wmat[l])
                w_sb.append(w)
            b_sb = constp.tile([DIMS, 3], f32, tag="bias")
            nc.sync.dma_start(out=b_sb[:], in_=bmat[:])
            ident = constp.tile([128, 128], f32, tag="ident")
            make_identity(nc, ident[:])

            # One-time Pool-engine touch of the idx tile: absorbs the
            # idx-load DMA wait so it is NOT embedded on the first
            # dma_gather (embedded cross-engine waits on the extended
            # gather opcode wedge the device).
            scratch = constp.tile([128, 1], i16, tag="scratch")
            nc.gpsimd.tensor_copy(out=scratch[:], in_=idx_sb[:, :1])

            # A single_packet gather is limited to 64 descriptors per SDMA
            # engine = 1024 indices; larger crashes the device.  Split each
            # chunk's gather into 8-block (1024-row) sub-gathers.
            GB = 8  # occurrence blocks per sub-gather

            for c in range(n_chunks):
                g = gatp.tile([128, kmax * DIMS], f32, tag="g")
                # Pool-engine touch of the dst slot: absorbs the slot-reuse
                # (WAR) wait for the same reason as above.
                nc.gpsimd.memset(g[:, :1], 0.0)
                for s in range(0, kmax, GB):
                    nb = min(GB, kmax - s)
                    n_sub = nb * 128
                    nc.gpsimd.dma_gather(
                        out_ap=g[:, s * DIMS:(s + nb) * DIMS].rearrange(
                            "p (k e) -> p k e", e=DIMS),
                        in_ap=table[c],
                        idxs_ap=idx_sb[:, c * fpc + s * 8:
                                       c * fpc + s * 8 + n_sub // 16],
                        num_idxs=n_sub,
                        num_idxs_reg=n_sub,
                        elem_size=DIMS,
                    )

                # segment sum: pairwise fold of the kmax occurrence blocks
                nb = kmax
                while nb > 1:
                    h = nb // 2
                    nc.vector.tensor_add(
                        out=g[:, : h * DIMS],
                        in0=g[:, : h * DIMS],
                        in1=g[:, (nb - h) * DIMS: nb * DIMS],
                    )
                    nb -= h

                # mean
                x = workp.tile([128, DIMS], f32, tag="x")
                nc.vector.tensor_scalar_mul(x[:], g[:, :DIMS], recip_sb[:, c:c + 1])

                # x^T
                xt_ps = psump.tile([DIMS, 128], f32, tag="xt")
                nc.tensor.transpose(out=xt_ps[:], in_=x[:], identity=ident[:])
                h_sb = workp.tile([DIMS, 128], f32, tag="h0")
                nc.scalar.activation(out=h_sb[:], in_=xt_ps[:],
                                     func=mybir.ActivationFunctionType.Copy)

                # y_l^T = relu(W_l^T h + b_l)   (all in transposed form)
                for l in range(3):
                    y_ps = psump.tile([DIMS, 128], f32, tag="y")
                    nc.tensor.matmul(out=y_ps[:], lhsT=w_sb[l][:], rhs=h_sb[:],
                                     start=True, stop=True)
                    h_sb = workp.tile([DIMS, 128], f32, tag=f"h{l + 1}")
                    nc.scalar.activation(out=h_sb[:], in_=y_ps[:],
                                         func=mybir.ActivationFunctionType.Relu,
                                         bias=b_sb[:, l:l + 1])

                # transpose back and store
                y_out_ps = psump.tile([128, DIMS], f32, tag="yo")
                nc.tensor.transpose(out=y_out_ps[:], in_=h_sb[:],
                                    identity=ident[:DIMS, :DIMS])
                o_sb = workp.tile([128, DIMS], f32, tag="o")
                nc.vector.tensor_copy(out=o_sb[:], in_=y_out_ps[:])
                nc.sync.dma_start(out=out[c * SEG_TILE:(c + 1) * SEG_TILE, :],
                                  in_=o_sb[:])

    nc.compile()
    _NC_CACHE[meta] = nc
    return nc


# ----------------------------------------------------------------------------
# Entry points
# ----------------------------------------------------------------------------

def run(inputs, trace=False, tmpdir=None):
    """Build + run; returns (full_output [16384,64] f32, exec_time_ns|None)."""
    from concourse.bass_utils import run_bass_kernel_spmd

    in_maps, meta = _host_prep(**inputs)
    nc = _build_nc(meta)
    res = run_bass_kernel_spmd(nc, in_maps, core_ids=list(range(N_CORES)),
                               trace=trace, tmpdir=tmpdir)
    outs = [res.results[k]["out"] for k in range(N_CORES)]
    full = np.concatenate(outs, axis=0).astype(np.float32, copy=False)
    return full, res.exec_time_ns


def kernel(**inputs) -> np.ndarray:
    full, _ = run(inputs, trace=False)
    return full


# revision 7
# speedup vs baseline: 2.9218x; 2.9218x over previous
"""Trainium2 Bass kernel for segment-mean embedding-bag + 3-layer MLP.

Problem (hardcoded, from spec):
  emb_table [100000, 64] f32, feature_indices [819200] int, batch_indices
  [819200] int (sorted), W0..W2 [64,64], b0..b2 [64].
  out[s] = relu-MLP( mean_{i: batch_indices[i]==s} emb_table[feature_indices[i]] )

Strategy (8 NeuronCores, data-parallel over batch segments):
  - Each core owns 2048 contiguous segments (16 chunks x 128 segments).
  - Host prep (sharding): for each chunk, build a compact per-chunk table
    (unique rows referenced by that chunk, plus a zeros row for padding)
    and an int16 index list ordered so that gather position j = k*128 + p
    holds occurrence k of segment p.  dma_gather then lands segment p's
    rows in SBUF partition p, occurrences along the free axis.
  - Device: one dma_gather per chunk (128*KMAX rows of 256B), pairwise
    fold on DVE for the segment sum, multiply by 1/count, then the MLP on
    the tensor engine in transposed form (W_l stationary), ReLU+bias on
    the scalar engine, transpose back, DMA out.
"""

import numpy as np

VOCAB = 100000
DIMS = 64
B = 16384
N_CORES = 8
SEG_TILE = 128  # segments per chunk

_NC_CACHE: dict[tuple, object] = {}


# ----------------------------------------------------------------------------
# Host-side sharding / index preparation (numpy only)
# ----------------------------------------------------------------------------

def _host_prep(emb_table, W0, b0, W1, b1, W2, b2, feature_indices, batch_indices):
    emb = np.ascontiguousarray(np.asarray(emb_table, dtype=np.float32))
    fidx = np.asarray(feature_indices).astype(np.int64, copy=False)
    bidx = np.asarray(batch_indices).astype(np.int64, copy=False)
    nnz = fidx.shape[0]

    counts = np.bincount(bidx, minlength=B).astype(np.int64)
    starts = np.zeros(B + 1, dtype=np.int64)
    np.cumsum(counts, out=starts[1:])
    kmax = max(int(counts.max()), 1)

    # slot[s, k] = feature id of segment s's k-th occurrence, or -1 if k >= count
    ar = np.arange(kmax, dtype=np.int64)
    pos = starts[:-1, None] + np.minimum(ar[None, :], np.maximum(counts[:, None] - 1, 0))
    np.clip(pos, 0, max(nnz - 1, 0), out=pos)
    valid = ar[None, :] < counts[:, None]
    slot = np.where(valid, fidx[pos], -1)  # [B, kmax]

    b_loc = B // N_CORES
    n_chunks = b_loc // SEG_TILE
    npc = SEG_TILE * kmax          # gather rows per chunk
    fpc = npc // 16                # idx free-dim per chunk (int16 wrap)

    per_chunk = []  # (core, chunk) -> (rows [u,64] f32, idx16 [128, kmax])
    r_max = 0
    for core in range(N_CORES):
        for c in range(n_chunks):
            s0 = core * b_loc + c * SEG_TILE
            sm = slot[s0:s0 + SEG_TILE]  # [128, kmax]
            u, inv = np.unique(sm, return_inverse=True)
            inv = inv.reshape(sm.shape).astype(np.int16)
            if u[0] == -1:
                rows = np.empty((len(u), DIMS), dtype=np.float32)
                rows[0] = 0.0
                rows[1:] = emb[u[1:]]
            else:
                rows = emb[u]
            r_max = max(r_max, len(u))
            per_chunk.append((rows, inv))

    # stable table row-count across runs of the same input scale
    r_chunk = -(-r_max // 512) * 512

    in_maps = []
    wmat = np.ascontiguousarray(
        np.stack([W0, W1, W2]).astype(np.float32))          # [3, 64, 64]
    bmat = np.ascontiguousarray(
        np.stack([b0, b1, b2], axis=1).astype(np.float32))  # [64, 3]
    with np.errstate(divide="ignore"):
        recip_all = np.where(counts > 0, 1.0 / counts, np.inf).astype(np.float32)

    for core in range(N_CORES):
        table = np.zeros((n_chunks, r_chunk, DIMS), dtype=np.float32)
        idxs = np.empty((128, n_chunks * fpc), dtype=np.int16)
        for c in range(n_chunks):
            rows, inv = per_chunk[core * n_chunks + c]
            table[c, : len(rows)] = rows
            # position j = k*128 + p  ->  idx16[p, k];  wrap by 16, replicate x8
            arr = inv.T.ravel()                       # [npc]
            wrapped = arr.reshape(-1, 16).T           # [16, fpc]
            idxs[:, c * fpc:(c + 1) * fpc] = np.tile(wrapped, (8, 1))
        recip = np.ascontiguousarray(
            recip_all[core * b_loc:(core + 1) * b_loc].reshape(n_chunks, SEG_TILE).T
        )  # [128, n_chunks]
        in_maps.append({
            "table": table,
            "idxs": idxs,
            "recip": recip,
            "wmat": wmat,
            "bmat": bmat,
        })

    meta = (kmax, r_chunk, n_chunks)
    return in_maps, meta


# ----------------------------------------------------------------------------
# Bass program
# ----------------------------------------------------------------------------

def _build_nc(meta):
    if meta in _NC_CACHE:
        return _NC_CACHE[meta]

    import concourse.bacc as bacc
    import concourse.tile as tile
    from concourse import mybir
    from concourse.masks import make_identity

    kmax, r_chunk, n_chunks = meta
    npc = SEG_TILE * kmax
    fpc = npc // 16
    f32 = mybir.dt.float32
    i16 = mybir.dt.int16

    nc = bacc.Bacc("TRN2", target_bir_lowering=False, debug=False,
                   enable_asserts=False, num_devices=N_CORES,
                   num_swdge_queues=4)

    table = nc.dram_tensor("table", [n_chunks, r_chunk, DIMS], f32, kind="ExternalInput")
    idxs = nc.dram_tensor("idxs", [128, n_chunks * fpc], i16, kind="ExternalInput")
    recip = nc.dram_tensor("recip", [128, n_chunks], f32, kind="ExternalInput")
    wmat = nc.dram_tensor("wmat", [3, DIMS, DIMS], f32, kind="ExternalInput")
    bmat = nc.dram_tensor("bmat", [DIMS, 3], f32, kind="ExternalInput")
    out = nc.dram_tensor("out", [n_chunks * SEG_TILE, DIMS], f32, kind="ExternalOutput")

    with tile.TileContext(nc) as tc:
        with tc.tile_pool(name="const", bufs=1) as constp, \
             tc.tile_pool(name="gat", bufs=4) as gatp, \
             tc.tile_pool(name="work", bufs=2) as workp, \
             tc.tile_pool(name="ps", bufs=2, space="PSUM") as psump:

            idx_sb = constp.tile([128, n_chunks * fpc], i16, tag="idx")
            nc.sync.dma_start(out=idx_sb[:], in_=idxs[:])
            recip_sb = constp.tile([128, n_chunks], f32, tag="recip")
            nc.sync.dma_start(out=recip_sb[:], in_=recip[:])
            w_sb = []
            for l in range(3):
                w = constp.tile([DIMS, DIMS], f32, tag=f"w{l}")
                nc.sync.dma_start(out=w[:], in_=wmat[l])
                w_sb.append(w)
            b_sb = constp.tile([DIMS, 3], f32, tag="bias")
            nc.sync.dma_start(out=b_sb[:], in_=bmat[:])
            ident = constp.tile([128, 128], f32, tag="ident")
            make_identity(nc, ident[:])

            # One-time Pool-engine touch of the idx tile: absorbs the
            # idx-load DMA wait so it is NOT embedded on the first
            # dma_gather (embedded cross-engine waits on the extended
            # gather opcode wedge the device).
            scratch = constp.tile([128, 1], i16, tag="scratch")
            nc.gpsimd.tensor_copy(out=scratch[:], in_=idx_sb[:, :1])

            # A single_packet gather is limited to 64 descriptors per SDMA
            # engine = 1024 indices; larger crashes the device.  Split each
            # chunk's gather into 8-block (1024-row) sub-gathers.
            GB = 8  # occurrence blocks per sub-gather
            rr = 0  # round-robin across the 4 SWDGE queues (Q7 core pairs)

            for c in range(n_chunks):
                g = gatp.tile([128, kmax * DIMS], f32, tag="g")
                # Pool-engine touch of the dst slot: absorbs the slot-reuse
                # (WAR) wait for the same reason as above.
                nc.gpsimd.memset(g[:, :1], 0.0)
                for s in range(0, kmax, GB):
                    nb = min(GB, kmax - s)
                    n_sub = nb * 128
                    nc.gpsimd.dma_gather(
                        out_ap=g[:, s * DIMS:(s + nb) * DIMS].rearrange(
                            "p (k e) -> p k e", e=DIMS),
                        in_ap=table[c],
                        idxs_ap=idx_sb[:, c * fpc + s * 8:
                                       c * fpc + s * 8 + n_sub // 16],
                        num_idxs=n_sub,
                        num_idxs_reg=n_sub,
                        elem_size=DIMS,
                        queue_num=rr % 4,
                    )
                    rr += 1

                # segment sum: pairwise fold of the kmax occurrence blocks
                nb = kmax
                while nb > 1:
                    h = nb // 2
                    nc.vector.tensor_add(
                        out=g[:, : h * DIMS],
                        in0=g[:, : h * DIMS],
                        in1=g[:, (nb - h) * DIMS: nb * DIMS],
                    )
                    nb -= h

                # mean
                x = workp.tile([128, DIMS], f32, tag="x")
                nc.vector.tensor_scalar_mul(x[:], g[:, :DIMS], recip_sb[:, c:c + 1])

                # x^T
                xt_ps = psump.tile([DIMS, 128], f32, tag="xt")
                nc.tensor.transpose(out=xt_ps[:], in_=x[:], identity=ident[:])
                h_sb = workp.tile([DIMS, 128], f32, tag="h0")
                nc.scalar.activation(out=h_sb[:], in_=xt_ps[:],
                                     func=mybir.ActivationFunctionType.Copy)

                # y_l^T = relu(W_l^T h + b_l)   (all in transposed form)
                for l in range(3):
                    y_ps = psump.tile([DIMS, 128], f32, tag="y")
                    nc.tensor.matmul(out=y_ps[:], lhsT=w_sb[l][:], rhs=h_sb[:],
                                     start=True, stop=True)
                    h_sb = workp.tile([DIMS, 128], f32, tag=f"h{l + 1}")
                    nc.scalar.activation(out=h_sb[:], in_=y_ps[:],
                                         func=mybir.ActivationFunctionType.Relu,
                                         bias=b_sb[:, l:l + 1])

                # transpose back and store
                y_out_ps = psump.tile([128, DIMS], f32, tag="yo")
                nc.tensor.transpose(out=y_out_ps[:], in_=h_sb[:],
                                    identity=ident[:DIMS, :DIMS])
                o_sb = workp.tile([128, DIMS], f32, tag="o")
                nc.vector.tensor_copy(out=o_sb[:], in_=y_out_ps[:])
                nc.sync.dma_start(out=out[c * SEG_TILE:(c + 1) * SEG_TILE, :],
                                  in_=o_sb[:])

    nc.compile()
    _NC_CACHE[meta] = nc
    return nc


# ----------------------------------------------------------------------------
# Entry points
# ----------------------------------------------------------------------------

def run(inputs, trace=False, tmpdir=None):
    """Build + run; returns (full_output [16384,64] f32, exec_time_ns|None)."""
    from concourse.bass_utils import run_bass_kernel_spmd

    in_maps, meta = _host_prep(**inputs)
    nc = _build_nc(meta)
    res = run_bass_kernel_spmd(nc, in_maps, core_ids=list(range(N_CORES)),
                               trace=trace, tmpdir=tmpdir)
    outs = [res.results[k]["out"] for k in range(N_CORES)]
    full = np.concatenate(outs, axis=0).astype(np.float32, copy=False)
    return full, res.exec_time_ns


def kernel(**inputs) -> np.ndarray:
    full, _ = run(inputs, trace=False)
    return full
